# revision 1
# baseline (speedup 1.0000x reference)
"""EnsembleFC (E=16 MLPs, 512->512->512->1, relu) on 8 TRN2 NeuronCores.

Strategy (expert parallel): each core owns E/8 = 2 ensemble members' weights
and computes their [B] output column; x is replicated. All activations stay
in "feature-major" (transposed) layout so no on-device transposes are needed:

    h1^T = relu(W1^T @ x^T + b1)      [H, B]
    h2^T = relu(W2^T @ h1^T + b2)     [H, B]
    out^T = W3^T @ h2^T               [1, B]   (b3 added on host)

Matmuls run in float32r (TRN2 reduced-precision fp32 PE mode, 1 cycle/row --
4x faster than plain fp32, ~20x more accurate than bf16; measured scaled
error ~1.5e-4 per 128-deep contraction with raw fp32 inputs).

Raw Bass (no Tile framework): this container's walrus rejects instructions
with more than a couple of sync waits, which Tile's auto-generated drains
exceed. Explicit per-engine programs with standalone waits keep every
instruction at one wait.

Pipeline per chunk of 512 batch columns (PE order, software-pipelined):
  ... L1(c, interleaved members) L3(c-1,m0) L3(c-1,m1) L2(c, interleaved) ...
  Members' output-tile groups interleave in pairs (ORDER) so the relu that
  recycles one member's psum banks lands while the other member's groups
  occupy the PE -- doubles the bank-drain slack.
  PSUM: each member-layer pair owns 2 banks (mt % 2 rotation); L3 reuses the
       member's second L2 bank at partition 0.
  ACT: relu+bias drains psum into h1/h2 (f32r).
  DVE: reduces h2 over k-tiles with the w3 weights in exact fp32
       (t_r = sum_kt w3[kt] * h2[kt], rounded to f32r at the end), so L3 is a
       single ones-vector matmul per member-chunk instead of four; also
       copies L3 psum rows to the output staging buffer.
  SP:  weight DMAs (per-tensor sems, split per k-tile), x chunk DMAs
       (per-slot sems -- DMA queue completions are unordered), output stores.
A short burst of dummy matmuls on scratch SBUF during the DMA prologue keeps
the PE HAM clock-gate warm so chunk 0 runs at full clock.
"""
import numpy as np

E, D, H, B = 16, 512, 512, 8192
N_CORES = 8
MPC = E // N_CORES          # members per core
KT = D // 128               # k-tiles per 512 contraction
MT = H // 128               # m-tiles per 512 output dim
CH = 512                    # batch columns per chunk (one psum bank)
NCH = B // CH               # chunks
XBUF = 4                    # x chunk buffering

_CACHE = {}


def _build():
    import concourse.bass as bass
    from concourse import mybir

    f32 = mybir.dt.float32
    f32r = mybir.dt.float32r

    nc = bass.Bass("TRN2", target_bir_lowering=False, debug=False,
                   num_devices=N_CORES)

    xT = nc.dram_tensor("xT", [D, B], f32r, kind="ExternalInput").ap()
    w1 = nc.dram_tensor("w1", [MPC, D, H], f32r, kind="ExternalInput").ap()
    w2 = nc.dram_tensor("w2", [MPC, H, H], f32r, kind="ExternalInput").ap()
    # host-side pre-arranged: w3[p, m, kt], b1/b2[p, m, mt]
    w3 = nc.dram_tensor("w3", [128, MPC, KT], f32r, kind="ExternalInput").ap()
    b1 = nc.dram_tensor("b1", [128, MPC, MT], f32, kind="ExternalInput").ap()
    b2 = nc.dram_tensor("b2", [128, MPC, MT], f32, kind="ExternalInput").ap()
    one = nc.dram_tensor("one", [128, 1], f32r, kind="ExternalInput").ap()
    out = nc.dram_tensor("out", [MPC, B], f32, kind="ExternalOutput").ap()

    w1s = [nc.alloc_sbuf_tensor(f"w1s{m}", [128, KT, H], f32r).ap()
           for m in range(MPC)]
    w2s = [nc.alloc_sbuf_tensor(f"w2s{m}", [128, KT, H], f32r).ap()
           for m in range(MPC)]
    w3s = nc.alloc_sbuf_tensor("w3s", [128, MPC, KT], f32r).ap()
    b1s = nc.alloc_sbuf_tensor("b1s", [128, MPC, MT], f32).ap()
    b2s = nc.alloc_sbuf_tensor("b2s", [128, MPC, MT], f32).ap()
    ones_s = nc.alloc_sbuf_tensor("ones_s", [128, 1], f32r).ap()
    xs = nc.alloc_sbuf_tensor("xs", [128, XBUF, KT, CH], f32r).ap()
    h1 = nc.alloc_sbuf_tensor("h1", [128, MPC, KT, CH], f32r).ap()
    h2 = nc.alloc_sbuf_tensor("h2", [128, MPC, KT, CH], f32r).ap()
    # DVE kt-reduction scratch (no aliasing: A,B pair-products, C,D partials)
    rA = nc.alloc_sbuf_tensor("rA", [128, CH], f32).ap()
    rB = nc.alloc_sbuf_tensor("rB", [128, CH], f32).ap()
    rC = nc.alloc_sbuf_tensor("rC", [128, CH], f32).ap()
    rD = nc.alloc_sbuf_tensor("rD", [128, CH], f32).ap()
    rE = nc.alloc_sbuf_tensor("rE", [128, CH], f32).ap()
    rF = nc.alloc_sbuf_tensor("rF", [128, CH], f32).ap()
    t_r = nc.alloc_sbuf_tensor("t_r", [128, MPC, CH], f32r).ap()
    # per-member output staging, both at partition 0
    osb = [nc.alloc_sbuf_tensor(f"osb{m}", [1, NCH, CH], f32).ap()
           for m in range(MPC)]

    psA = nc.alloc_psum_tensor("psA", [128, 2 * MPC, CH], f32).ap()  # L1
    psB = nc.alloc_psum_tensor("psB", [128, 2 * MPC, CH], f32).ap()  # L2+L3

    # PE warmup scratch: dummy matmuls during the DMA prologue keep the HAM
    # clock-gate ramp off the critical path (uninitialized on HW -- harmless)
    scr = nc.alloc_sbuf_tensor("scr", [128, 128 + CH], f32r).ap()
    N_WARM = _CACHE.get("n_warm_override", 28)

    xT_r = xT.rearrange("(kt p) b -> p kt b", p=128)

    # --- tick tables (absolute semaphore counts, mirror emission order) ---
    # members' groups interleave in pairs so the bank-drain relu of one
    # member lands while the other member's groups occupy the PE
    ORDER = [(0, 0), (0, 1), (1, 0), (1, 1), (0, 2), (0, 3), (1, 2), (1, 3)]
    mmT = {}
    _t = 0
    for c in range(NCH):
        for m, mt in ORDER:
            _t += 1
            mmT[("l1", c, m, mt)] = _t
        if c >= 1:
            for m in range(MPC):
                _t += 1
                mmT[("l3", c - 1, m)] = _t
        for m, mt in ORDER:
            _t += 1
            mmT[("l2", c, m, mt)] = _t
    for m in range(MPC):
        _t += 1
        mmT[("l3", NCH - 1, m)] = _t

    actT = {}
    _a = 0
    for c in range(NCH):
        for m, mt in ORDER:
            _a += 1
            actT[("r1", c, m, mt)] = _a
        for m, mt in ORDER:
            _a += 1
            actT[("r2", c, m, mt)] = _a

    def act_r1(c, m, mt):
        return actT[("r1", c, m, mt)]

    def act_r2(c, m, mt):
        return actT[("r2", c, m, mt)]

    # DVE tick table: per chunk red(m0), red(m1) [skipped for the last
    # chunk -- its L3 runs directly off h2], then cp(m0), cp(m1)
    dveT = {}
    _d = 0
    for c in range(NCH):
        if c < NCH - 1:
            for m in range(MPC):
                _d += 1
                dveT[("red", c, m)] = _d
        for m in range(MPC):
            _d += 1
            dveT[("cp", c, m)] = _d

    def dve_red(c, m):
        return dveT[("red", c, m)]

    def dve_cp(c, m):
        return dveT[("cp", c, m)]

    with (
        nc.Block() as block,
        nc.semaphore("mm_sem") as mm_sem,
        nc.semaphore("act_sem") as act_sem,
        nc.semaphore("b1_sem") as b1_sem,
        nc.semaphore("b2_sem") as b2_sem,
        nc.semaphore("w3_sem") as w3_sem,
        nc.semaphore("d_sem") as d_sem,
    ):
        # per-slot x semaphores: DMA queue completions are unordered across
        # chunks, so a single cumulative counter would be racy
        x_sems = [nc.alloc_semaphore(f"x_sem{s}") for s in range(XBUF)]
        dve_sem = nc.alloc_semaphore("dve_sem")
        rd_sem = nc.alloc_semaphore("rd_sem")   # intra-DVE RAW/WAR ordering
        w1_sems = [nc.alloc_semaphore(f"w1_sem{m}") for m in range(MPC)]
        w2_sems = [nc.alloc_semaphore(f"w2_sem{m}") for m in range(MPC)]
        # member-0 W1 arrives per output-tile: chunk 0's first groups start
        # after x0 + one 256KB weight slice instead of x0 + 1MB
        w1m_sems = [nc.alloc_semaphore(f"w1m_sem{t}") for t in range(MT)]

        def dma_x(sync, c):
            for kt in range(KT):
                sync.dma_start(
                    out=xs[:, c % XBUF, kt, :],
                    in_=xT_r[:, kt, c * CH:(c + 1) * CH],
                ).then_inc(x_sems[c % XBUF], 16)

        @block.sync
        def _(sync: bass.BassEngine):
            # interleave weight loads with early x chunks, ordered by need
            w1r = [w1[m].rearrange("(kt p) m2 -> p kt m2", p=128)
                   for m in range(MPC)]
            w2r = [w2[m].rearrange("(kt p) m2 -> p kt m2", p=128)
                   for m in range(MPC)]
            for mt in range(MT):
                sync.dma_start(
                    out=w1s[0][:, :, mt * 128:(mt + 1) * 128],
                    in_=w1r[0][:, :, mt * 128:(mt + 1) * 128],
                ).then_inc(w1m_sems[mt], 16)
            sync.dma_start(out=b1s, in_=b1).then_inc(b1_sem, 16)
            dma_x(sync, 0)
            for kt in range(KT):
                sync.dma_start(out=w1s[1][:, kt], in_=w1r[1][:, kt]
                               ).then_inc(w1_sems[1], 16)
            for kt in range(KT):
                sync.dma_start(out=w2s[0][:, kt], in_=w2r[0][:, kt]
                               ).then_inc(w2_sems[0], 16)
            sync.dma_start(out=b2s, in_=b2).then_inc(b2_sem, 16)
            sync.dma_start(out=w3s, in_=w3).then_inc(w3_sem, 16)
            sync.dma_start(out=ones_s, in_=one).then_inc(w3_sem, 16)
            dma_x(sync, 1)
            for kt in range(KT):
                sync.dma_start(out=w2s[1][:, kt], in_=w2r[1][:, kt]
                               ).then_inc(w2_sems[1], 16)
            dma_x(sync, 2)
            dma_x(sync, 3)

            out_r = out.rearrange("m (nch ch) -> m nch ch", ch=CH)
            for c in range(XBUF, NCH):
                # x slot free once L1 of chunk c-XBUF fully consumed it
                sync.wait_ge(mm_sem, mmT[("l1", c - XBUF, MPC - 1, MT - 1)])
                dma_x(sync, c)
                # trailing store for chunk c-XBUF
                cs = c - XBUF
                sync.wait_ge(dve_sem, dve_cp(cs, MPC - 1))
                for m in range(MPC):
                    sync.dma_start(out=out_r[m:m + 1, cs],
                                   in_=osb[m][:, cs]).then_inc(d_sem, 16)

            for cs in range(NCH - XBUF, NCH):
                sync.wait_ge(dve_sem, dve_cp(cs, MPC - 1))
                for m in range(MPC):
                    sync.dma_start(out=out_r[m:m + 1, cs],
                                   in_=osb[m][:, cs]).then_inc(d_sem, 16)
            sync.wait_ge(d_sem, 16 * MPC * NCH)

        @block.vector
        def _(vector: bass.BassEngine):
            # DVE: (a) kt-reduction t_r = sum_kt w3[kt]*h2[kt] in exact fp32
            # (takes 3 of every 4 L3 matmuls off the PE, and is more accurate
            # than f32r products), (b) L3 psum -> osb copies.
            w3f = w3s.bitcast(f32)
            vector.wait_ge(w3_sem, 32)   # w3s + ones loaded
            for c in range(NCH):
                for m in range(MPC):
                    if c == NCH - 1:
                        break   # last chunk: PE computes L3 directly
                    # h2 ready; implies PE already read t_r(c-1, m) (its L3
                    # precedes this chunk's L2 in the PE stream)
                    h2f = h2[:, m].bitcast(f32)
                    # 4 independent muls, each gated on its own relu2, then a
                    # 2-level add tree: the reduction lands ~1 op after the
                    # LAST relu instead of a full serial chain after it.
                    # (DVE does not self-interlock; rd_sem orders RAW/WAR.)
                    base = 6 * (MPC * c + m)
                    if base:
                        vector.wait_ge(rd_sem, base)   # prev group's reads done
                    for kt, buf in enumerate((rA, rB, rC, rD)):
                        vector.wait_ge(act_sem, act_r2(c, m, kt))
                        vector.tensor_scalar_mul(
                            buf, h2f[:, kt, :], w3f[:, m, kt:kt + 1]
                        ).then_inc(rd_sem, 1)
                    vector.wait_ge(rd_sem, base + 2)
                    vector.tensor_add(rE, rA, rB).then_inc(rd_sem, 1)
                    vector.wait_ge(rd_sem, base + 4)
                    vector.tensor_add(rF, rC, rD).then_inc(rd_sem, 1)
                    vector.wait_ge(rd_sem, base + 6)
                    vector.tensor_add(t_r[:, m, :], rE, rF
                                      ).then_inc(dve_sem, 1)
                for m in range(MPC):
                    vector.wait_ge(mm_sem, mmT[("l3", c, m)])
                    vector.tensor_copy(
                        osb[m][0:1, c, :], psB[0:1, 2 * m + 1, :],
                    ).then_inc(dve_sem, 1)

        @block.tensor
        def _(tensor: bass.BassEngine):
            # warmup on uninitialized scratch: values are irrelevant, the psum
            # is overwritten (start=True) before any reader
            for i in range(N_WARM):
                tensor.matmul(psA[:, 0, :], scr[:, :128], scr[:, 128:],
                              start=True, stop=True, skip_group_check=True)

            def l3(c, m):
                # single ones-matmul over the DVE-reduced t_r; bank 2m+1 so
                # the osb copy only gates the SECOND L2 group of chunk c+1
                tensor.wait_ge(dve_sem, dve_red(c, m))
                tensor.matmul(
                    psB[0:1, 2 * m + 1, :], ones_s, t_r[:, m, :],
                    start=True, stop=True,
                ).then_inc(mm_sem, 1)

            for c in range(NCH):
                tensor.wait_ge(x_sems[c % XBUF], 64 * (c // XBUF + 1))
                # L1, members interleaved
                for m, mt in ORDER:
                    if mt == 0:
                        if c == 0:
                            if m == 1:
                                tensor.wait_ge(w1_sems[1], 64)
                        else:
                            # banks 2m,2m+1 drained by chunk c-1's L1 relus
                            tensor.wait_ge(act_sem, act_r1(c - 1, m, MT - 1))
                    if c == 0 and m == 0:
                        tensor.wait_ge(w1m_sems[mt], 16)
                    if mt >= 2:               # 2-bank rotation WAR
                        tensor.wait_ge(act_sem, act_r1(c, m, mt - 2))
                    for kt in range(KT):
                        ins = tensor.matmul(
                            psA[:, 2 * m + mt % 2, :],
                            w1s[m][:, kt, mt * 128:(mt + 1) * 128],
                            xs[:, c % XBUF, kt, :],
                            start=(kt == 0), stop=(kt == KT - 1),
                        )
                    ins.then_inc(mm_sem, 1)
                # pipelined L3 of the previous chunk: its DVE reduction ran
                # while this chunk's L1 was on the PE
                if c >= 1:
                    if c == 1:
                        tensor.wait_ge(w3_sem, 32)
                    for m in range(MPC):
                        l3(c - 1, m)
                # L2, members interleaved
                for m, mt in ORDER:
                    if mt == 0:
                        if c == 0:
                            tensor.wait_ge(w2_sems[m], 64)
                        tensor.wait_ge(act_sem, act_r1(c, m, MT - 1))  # h1
                    if mt == 1 and c > 0:
                        # psB bank 2m+1 holds chunk c-1's L3 row until DVE
                        # copies it out
                        tensor.wait_ge(dve_sem, dve_cp(c - 1, m))
                    if mt >= 2:
                        tensor.wait_ge(act_sem, act_r2(c, m, mt - 2))
                    for kt in range(KT):
                        ins = tensor.matmul(
                            psB[:, 2 * m + mt % 2, :],
                            w2s[m][:, kt, mt * 128:(mt + 1) * 128],
                            h1[:, m, kt, :],
                            start=(kt == 0), stop=(kt == KT - 1),
                        )
                    ins.then_inc(mm_sem, 1)
            # tail: direct w3 matmuls for the last chunk -- avoids idling on
            # the serial DVE reduction after the final L2
            for m in range(MPC):
                tensor.wait_ge(act_sem, act_r2(NCH - 1, m, MT - 1))
                for kt in range(KT):
                    ins = tensor.matmul(
                        psB[0:1, 2 * m + 1, :],
                        w3s[:, m, kt:kt + 1],
                        h2[:, m, kt, :],
                        start=(kt == 0), stop=(kt == KT - 1),
                    )
                ins.then_inc(mm_sem, 1)

        @block.scalar
        def _(scalar: bass.BassEngine):
            Relu = bass.mybir.ActivationFunctionType.Relu
            scalar.wait_ge(b1_sem, 16)
            scalar.wait_ge(b2_sem, 16)
            for c in range(NCH):
                for m, mt in ORDER:
                    scalar.wait_ge(mm_sem, mmT[("l1", c, m, mt)])
                    scalar.activation(
                        h1[:, m, mt, :], psA[:, 2 * m + mt % 2, :], Relu,
                        bias=b1s[:, m, mt:mt + 1],
                    ).then_inc(act_sem, 1)
                for m, mt in ORDER:
                    scalar.wait_ge(mm_sem, mmT[("l2", c, m, mt)])
                    scalar.activation(
                        h2[:, m, mt, :], psB[:, 2 * m + mt % 2, :], Relu,
                        bias=b2s[:, m, mt:mt + 1],
                    ).then_inc(act_sem, 1)

    return nc


def get_nc():
    if "nc" not in _CACHE:
        _CACHE["nc"] = _build()
    return _CACHE["nc"]


def kernel(x, W1, b1, W2, b2, W3, b3):
    from concourse.bass_utils import run_bass_kernel_spmd

    nc = get_nc()
    xT = np.ascontiguousarray(np.asarray(x, dtype=np.float32).T)
    W1 = np.asarray(W1, dtype=np.float32)
    W2 = np.asarray(W2, dtype=np.float32)
    W3 = np.asarray(W3, dtype=np.float32)
    b1 = np.asarray(b1, dtype=np.float32)
    b2 = np.asarray(b2, dtype=np.float32)
    b3 = np.asarray(b3, dtype=np.float32)

    def feat_major(v):
        # [MPC, H] -> [128, MPC, H//128]: v[p, m, t] = v_in[m, t*128 + p]
        return np.ascontiguousarray(
            v.reshape(MPC, H // 128, 128).transpose(2, 0, 1))

    in_maps = []
    for c in range(N_CORES):
        s = slice(MPC * c, MPC * (c + 1))
        in_maps.append({
            "xT": xT,
            "w1": np.ascontiguousarray(W1[s]),
            "w2": np.ascontiguousarray(W2[s]),
            "w3": feat_major(W3[s, :, 0]),
            "b1": feat_major(b1[s]),
            "b2": feat_major(b2[s]),
            "one": np.ones((128, 1), dtype=np.float32),
        })

    res = run_bass_kernel_spmd(nc, in_maps, list(range(N_CORES)))
    out = np.concatenate([r["out"] for r in res.results], axis=0)  # [E, B]
    out = out + b3.reshape(E, 1)
    return out.reshape(E, B, 1).astype(np.float32)



# revision 2
# speedup vs baseline: 1.0028x; 1.0028x over previous
"""EnsembleFC (E=16 MLPs, 512->512->512->1, relu) on 8 TRN2 NeuronCores.

Expert parallel: each core owns E/8 = 2 members' weights and computes their
[B] output column; x replicated. Feature-major (transposed) activations:

    h1^T = relu(W1^T @ x^T + b1)      [H, B]
    h2^T = relu(W2^T @ h1^T + b2)     [H, B]
    out^T = W3^T @ h2^T               [1, B]   (b3 added on host)

L1/L2 run on the PE in fp8e4m3 DoubleRow perf mode (2 k-slots of 128 per
instruction at 0.5 cycles/moving-column -- 4x the f32r slot rate). Full
f32-level accuracy is kept with hi/lo splits: a = a_hi + a_lo (both fp8),
  a @ w = a_hi@w_hi + a_hi@w_lo + a_lo@w_hi   (+ a_lo@w_lo, dropped ~1e-4)
3 products at 4x rate = 1.33x over f32r (measured scaled err ~2e-3).
Scales keep fp8 in normal range: x,h1 carried at 16x, W1,W2 at 64x; the
act engine rescales while applying relu+bias. Per output tile the 6
DoubleRow matmuls are: hh(kt01), hh(kt23), then per-kt cross instructions
whose two slots pack (w_lo,a_hi)+(w_hi,a_lo).

x and W splits happen on host. h1's split is on-device, pipelined so the
PE never stalls (a PE gap costs ~3us of clock ramp in the p-state model):
  ACT:    h1f = relu(psA/64 + 16*b1) f32; h2 = relu(psB/1024 + b2) fp16
  DVE:    m0 hi=fp8(h1f), lo=fp8(h1f-hi); m1 lo; and the w3 reduction
          t_r[p,:] = sum_kt w3[p,kt]*h2[p,kt,:] as a mul/add tree in fp16
          (fp16 gets the DVE 2x/4x modes; scalar_tensor_tensor gets none)
  GPSIMD: m1 hi casts (all it supports: no PSUM access, tensor ops only)
L3: t_r partial sums are DMA'd straight to DRAM; the host finishes the
128-partition reduction (0.01% of the FLOPs). This keeps the whole L3
off the PE: a chunk is exactly 96 DoubleRow matmuls = 10.27us, and the
PE never waits on the reduction chain.

PSUM banks are mapped by output tile (bank=mt, psA for L1 / psB for L2),
so the only accumulation-bank WAR is against the other member's act
drain, half a chunk away. Per-chunk engine budgets: PE 10.27us, ACT 9.7,
DVE 9.6, GPSIMD 3.1.

Raw Bass (one wait per instruction), absolute semaphore tick tables.
"""
import numpy as np
import ml_dtypes

F8 = ml_dtypes.float8_e4m3

E, D, H, B = 16, 512, 512, 8192
N_CORES = 8
MPC = E // N_CORES          # members per core
KT = D // 128               # k-tiles per 512 contraction
MT = H // 128               # m-tiles per 512 output dim
CH = 512                    # batch columns per chunk (one psum bank)
NCH = B // CH               # chunks
XBUF = 4                    # x chunk buffering

SX = 16.0                   # x / h1 fp8 carry scale
SW = 64.0                   # W1 / W2 fp8 carry scale

_CACHE = {}

# L1 group order: m0's tiles early (its h1 chain gates L2-m0 at ~5.3us),
# m1 interleaved so the per-tile act->DVE chains keep up.
ORDER_L1 = [(0, 0), (0, 1), (0, 2), (1, 0), (0, 3), (1, 1), (1, 2), (1, 3)]
# L2 emission: all m0 then all m1 (m1's lo tiles land ~7.8us).
ORDER_L2 = [(0, 0), (0, 1), (0, 2), (0, 3), (1, 0), (1, 1), (1, 2), (1, 3)]


def _build():
    import concourse.bass as bass
    from concourse import mybir

    f32 = mybir.dt.float32
    f16 = mybir.dt.float16
    f8 = mybir.dt.float8e4
    DR = mybir.MatmulPerfMode.DoubleRow
    Relu = mybir.ActivationFunctionType.Relu

    nc = bass.Bass("TRN2", target_bir_lowering=False, debug=False,
                   num_devices=N_CORES)

    # dram (host pre-split/scaled; streams: x/h1 [hi,lo], w [lo,hi])
    xd = nc.dram_tensor("xd", [128, NCH, 2 * KT * CH], f8,
                        kind="ExternalInput").ap()
    w1 = [nc.dram_tensor(f"w1_{m}", [128, 2, KT, H], f8,
                         kind="ExternalInput").ap() for m in range(MPC)]
    w2 = [nc.dram_tensor(f"w2_{m}", [128, 2, KT, H], f8,
                         kind="ExternalInput").ap() for m in range(MPC)]
    w3 = nc.dram_tensor("w3", [128, MPC, KT], f32, kind="ExternalInput").ap()
    b1 = nc.dram_tensor("b1", [128, MPC, MT], f32, kind="ExternalInput").ap()
    b2 = nc.dram_tensor("b2", [128, MPC, MT], f32, kind="ExternalInput").ap()
    trd = nc.dram_tensor("trd", [128, NCH, MPC, CH], f16,
                         kind="ExternalOutput").ap()

    # sbuf
    w1s = [nc.alloc_sbuf_tensor(f"w1s{m}", [128, 2, KT, H], f8).ap()
           for m in range(MPC)]
    w2s = [nc.alloc_sbuf_tensor(f"w2s{m}", [128, 2, KT, H], f8).ap()
           for m in range(MPC)]
    w3s = nc.alloc_sbuf_tensor("w3s", [128, MPC, KT], f32).ap()
    b1s = nc.alloc_sbuf_tensor("b1s", [128, MPC, MT], f32).ap()
    b2s = nc.alloc_sbuf_tensor("b2s", [128, MPC, MT], f32).ap()
    xs = nc.alloc_sbuf_tensor("xs", [128, XBUF, 2, KT, CH], f8).ap()
    h1f = nc.alloc_sbuf_tensor("h1f", [128, MPC, MT, CH], f32).ap()
    h18 = nc.alloc_sbuf_tensor("h18", [128, MPC, 2, KT, CH], f8).ap()
    h2 = nc.alloc_sbuf_tensor("h2", [128, MPC, KT, CH], f16).ap()
    rP = [nc.alloc_sbuf_tensor(f"rP{m}", [128, KT, CH], f16).ap()
          for m in range(MPC)]
    rE = [nc.alloc_sbuf_tensor(f"rE{m}", [128, CH], f16).ap()
          for m in range(MPC)]
    rF = [nc.alloc_sbuf_tensor(f"rF{m}", [128, CH], f16).ap()
          for m in range(MPC)]
    t_r = nc.alloc_sbuf_tensor("t_r", [128, MPC, CH], f16).ap()

    psA = nc.alloc_psum_tensor("psA", [128, MT, CH], f32).ap()  # L1
    psB = nc.alloc_psum_tensor("psB", [128, MT, CH], f32).ap()  # L2

    # PE warmup scratch (uninitialized; the p-state model needs ~3us of
    # continuous execution to reach full clock)
    scr = nc.alloc_sbuf_tensor("scr", [128, 128 + CH],
                               mybir.dt.float32r).ap()
    N_WARM = _CACHE.get("n_warm_override", 8)
    N_WARM2 = _CACHE.get("n_warm2_override", 0)

    # --- tick tables (absolute counts, mirror emission order) ---
    mmT = {}
    _t = 0
    for c in range(NCH):
        for m, mt in ORDER_L1:
            _t += 1
            mmT[("l1", c, m, mt)] = _t
        for m, mt in ORDER_L2:
            _t += 1
            mmT[("l2", c, m, mt)] = _t

    actT = {}
    _a = 0
    for c in range(NCH):
        for m, mt in ORDER_L1:
            _a += 1
            actT[("r1", c, m, mt)] = _a
        for m, mt in ORDER_L2:
            _a += 1
            actT[("r2", c, m, mt)] = _a

    dveT = {}
    _d = 0
    for c in range(NCH):
        for kt in range(KT):
            _d += 1
            dveT[("hi", c, 0, kt)] = _d
            _d += 1
            dveT[("lo", c, 0, kt)] = _d
        for kt in range(KT):
            _d += 1
            dveT[("lo", c, 1, kt)] = _d
        for m in range(MPC):
            # mul0, mul1, addE, mul2, mul3, addF, t_r
            _d += 7
            dveT[("red", c, m)] = _d

    poolT = {}
    _p = 0
    for c in range(NCH):
        for kt in range(KT):
            _p += 1
            poolT[("hi", c, 1, kt)] = _p

    with (
        nc.Block() as block,
        nc.semaphore("mm_sem") as mm_sem,
        nc.semaphore("act_sem") as act_sem,
        nc.semaphore("b1_sem") as b1_sem,
        nc.semaphore("b2_sem") as b2_sem,
        nc.semaphore("w3_sem") as w3_sem,
        nc.semaphore("d_sem") as d_sem,
    ):
        x_sems = [nc.alloc_semaphore(f"x_sem{s}") for s in range(XBUF)]
        dve_sem = nc.alloc_semaphore("dve_sem")
        pool_sem = nc.alloc_semaphore("pool_sem")
        w1_sems = [nc.alloc_semaphore(f"w1_sem{m}") for m in range(MPC)]
        w2_sems = [nc.alloc_semaphore(f"w2_sem{m}") for m in range(MPC)]

        xs_flat = xs.rearrange("p s a b c -> p s (a b c)")

        def dma_x(sync, c):
            # whole chunk (both streams, all kt) in one contiguous DMA
            sync.dma_start(
                out=xs_flat[:, c % XBUF, :],
                in_=xd[:, c, :],
            ).then_inc(x_sems[c % XBUF], 16)

        @block.sync
        def _(sync: bass.BassEngine):
            def wflat(ap):
                return ap.rearrange("p s k h -> p (s k h)")

            # SP queue: member-0 W1 + x chunks; the rest loads in parallel
            # on the Activation queue. (One DMA each: the HWDGE descriptor
            # stage serializes at ~0.6us per DMA.)
            sync.dma_start(out=wflat(w1s[0]), in_=wflat(w1[0])
                           ).then_inc(w1_sems[0], 16)
            dma_x(sync, 0)
            dma_x(sync, 1)
            dma_x(sync, 2)
            dma_x(sync, 3)

            def dma_tr(cs, m):
                sync.wait_ge(dve_sem, dveT[("red", cs, m)])
                sync.dma_start(out=trd[:, cs, m, :], in_=t_r[:, m, :]
                               ).then_inc(d_sem, 16)

            for c in range(NCH):
                if c >= 1:
                    dma_tr(c - 1, 0)
                if c + XBUF < NCH:
                    lm, lmt = ORDER_L1[-1]
                    sync.wait_ge(mm_sem, mmT[("l1", c, lm, lmt)])
                    dma_x(sync, c + XBUF)
                if c >= 1:
                    dma_tr(c - 1, 1)
            for m in range(MPC):
                dma_tr(NCH - 1, m)
            sync.wait_ge(d_sem, 16 * MPC * NCH)

        @block.tensor
        def _(tensor: bass.BassEngine):
            for i in range(N_WARM):
                tensor.matmul(psA[:, 0, :], scr[:, :128], scr[:, 128:],
                              start=True, stop=True, skip_group_check=True)

            def dr_group(ps_bank, wsrc, asrc_hh, asrc_cr, waits=None):
                """6 DoubleRow matmuls accumulating one [128, CH] tile.

                waits: optional dict instr_idx -> (sem, tick) emitted
                before that instruction (0=hh01, 1=hh23, 2..5=cr kt)."""
                ins = None
                for idx in range(6):
                    if waits and idx in waits:
                        sem, tick = waits[idx]
                        tensor.wait_ge(sem, tick)
                    if idx < 2:
                        kt = 2 * idx
                        ins = tensor.matmul(
                            ps_bank, wsrc(1, slice(kt, kt + 2)),
                            asrc_hh(slice(kt, kt + 2)),
                            start=(idx == 0), stop=False, perf_mode=DR)
                    else:
                        kt = idx - 2
                        ins = tensor.matmul(
                            ps_bank, wsrc(slice(0, 2), kt), asrc_cr(kt),
                            start=False, stop=(idx == 5), perf_mode=DR)
                return ins

            for c in range(NCH):
                tensor.wait_ge(x_sems[c % XBUF], 16 * (c // XBUF + 1))
                for m, mt in ORDER_L1:
                    if c == 0:
                        if mt == 0:
                            tensor.wait_ge(w1_sems[m], 16)
                    else:
                        # psA bank=mt WAR vs the other member's r1 drain
                        om = 1 - m
                        cc = c - 1 if m == 0 else c
                        tensor.wait_ge(act_sem, actT[("r1", cc, om, mt)])
                    dr_group(
                        psA[:, mt, :],
                        lambda st, ktsl, m=m, cs=slice(mt * 128, (mt + 1) * 128):
                            w1s[m][:, st, ktsl, cs],
                        lambda ktsl, c=c: xs[:, c % XBUF, 0, ktsl, :],
                        lambda kt, c=c: xs[:, c % XBUF, :, kt, :],
                    ).then_inc(mm_sem, 1)

                if c == 0:
                    # keep the PE clock hot while the act/DVE pipeline
                    # fills for the first L2 phase
                    for i in range(N_WARM2):
                        tensor.matmul(psB[:, 0, :], scr[:, :128],
                                      scr[:, 128:], start=True, stop=True,
                                      skip_group_check=True)
                for m, mt in ORDER_L2:
                    waits = None
                    if mt == 0:
                        if c == 0:
                            tensor.wait_ge(w2_sems[m], 16)
                        if m == 0:
                            waits = {
                                0: (dve_sem, dveT[("hi", c, 0, 1)]),
                                1: (dve_sem, dveT[("hi", c, 0, 3)]),
                                **{2 + kt: (dve_sem, dveT[("lo", c, 0, kt)])
                                   for kt in range(KT)},
                            }
                        else:
                            waits = {
                                0: (pool_sem, poolT[("hi", c, 1, 1)]),
                                1: (pool_sem, poolT[("hi", c, 1, 3)]),
                                **{2 + kt: (dve_sem, dveT[("lo", c, 1, kt)])
                                   for kt in range(KT)},
                            }
                    # psB bank=mt WAR vs the other member's r2 drain
                    if m == 0 and c >= 1:
                        tensor.wait_ge(act_sem, actT[("r2", c - 1, 1, mt)])
                    elif m == 1:
                        tensor.wait_ge(act_sem, actT[("r2", c, 0, mt)])
                    dr_group(
                        psB[:, mt, :],
                        lambda st, ktsl, m=m, cs=slice(mt * 128, (mt + 1) * 128):
                            w2s[m][:, st, ktsl, cs],
                        lambda ktsl, m=m: h18[:, m, 0, ktsl, :],
                        lambda kt, m=m: h18[:, m, :, kt, :],
                        waits=waits,
                    ).then_inc(mm_sem, 1)

        @block.scalar
        def _(scalar: bass.BassEngine):
            def wflat(ap):
                return ap.rearrange("p s k h -> p (s k h)")

            # prologue weight loads on the Activation HWDGE queue, in
            # parallel with SP's w1m0/x stream; tiny transfers first (the
            # DMA engine serializes transfers)
            scalar.dma_start(out=b1s, in_=b1).then_inc(b1_sem, 16)
            scalar.dma_start(out=b2s, in_=b2).then_inc(b2_sem, 16)
            scalar.dma_start(out=w3s, in_=w3).then_inc(w3_sem, 16)
            scalar.dma_start(out=wflat(w1s[1]), in_=wflat(w1[1])
                             ).then_inc(w1_sems[1], 16)
            scalar.dma_start(out=wflat(w2s[0]), in_=wflat(w2[0])
                             ).then_inc(w2_sems[0], 16)
            scalar.dma_start(out=wflat(w2s[1]), in_=wflat(w2[1])
                             ).then_inc(w2_sems[1], 16)
            scalar.wait_ge(b1_sem, 16)
            scalar.wait_ge(b2_sem, 16)
            for c in range(NCH):
                for m, mt in ORDER_L1:
                    if mt == 0 and c > 0:
                        # h1f[m] WAR: last lo pass of chunk c-1 done
                        scalar.wait_ge(dve_sem,
                                       dveT[("lo", c - 1, m, MT - 1)])
                    scalar.wait_ge(mm_sem, mmT[("l1", c, m, mt)])
                    # h1f = relu(psum/SW + SX*b1) = SX * h1_true
                    scalar.activation(
                        h1f[:, m, mt, :], psA[:, mt, :], Relu,
                        bias=b1s[:, m, mt:mt + 1], scale=1.0 / SW,
                    ).then_inc(act_sem, 1)
                for m, mt in ORDER_L2:
                    if mt == 0 and c > 0:
                        # h2[m] WAR: chunk c-1's reduction read it
                        scalar.wait_ge(dve_sem, dveT[("red", c - 1, m)])
                    scalar.wait_ge(mm_sem, mmT[("l2", c, m, mt)])
                    scalar.activation(
                        h2[:, m, mt, :], psB[:, mt, :], Relu,
                        bias=b2s[:, m, mt:mt + 1], scale=1.0 / (SX * SW),
                    ).then_inc(act_sem, 1)

        @block.vector
        def _(vector: bass.BassEngine):
            for c in range(NCH):
                # m0: hi + lo pairs per tile
                for kt in range(KT):
                    if kt == 0 and c > 0:
                        # h18[m0] WAR: PE read it for chunk c-1's L2
                        vector.wait_ge(mm_sem, mmT[("l2", c - 1, 0, MT - 1)])
                    vector.wait_ge(act_sem, actT[("r1", c, 0, kt)])
                    vector.tensor_copy(h18[:, 0, 0, kt, :], h1f[:, 0, kt, :]
                                       ).then_inc(dve_sem, 1)
                    vector.tensor_sub(h18[:, 0, 1, kt, :], h1f[:, 0, kt, :],
                                      h18[:, 0, 0, kt, :]).then_inc(dve_sem, 1)
                # m1: lo only (hi on gpsimd)
                for kt in range(KT):
                    vector.wait_ge(pool_sem, poolT[("hi", c, 1, kt)])
                    vector.tensor_sub(h18[:, 1, 1, kt, :], h1f[:, 1, kt, :],
                                      h18[:, 1, 0, kt, :]).then_inc(dve_sem, 1)
                # w3 reduction as fp16 muls (4x mode) + add tree (2x mode):
                # t_r[p,:] = sum_kt w3[p,kt] * h2[p,kt,:]
                if c == 0:
                    vector.wait_ge(w3_sem, 16)
                for m in range(MPC):
                    for kt in range(KT):
                        vector.wait_ge(act_sem, actT[("r2", c, m, kt)])
                        vector.tensor_scalar_mul(
                            rP[m][:, kt, :], h2[:, m, kt, :],
                            w3s[:, m, kt:kt + 1]).then_inc(dve_sem, 1)
                        if kt == 1:
                            vector.tensor_add(
                                rP[m][:, 0, :], rP[m][:, 0, :], rP[m][:, 1, :]
                            ).then_inc(dve_sem, 1)
                        if kt == 3:
                            vector.tensor_add(
                                rP[m][:, 2, :], rP[m][:, 2, :], rP[m][:, 3, :]
                            ).then_inc(dve_sem, 1)
                    if c >= 1:
                        # t_r[m] WAR vs its DMA of chunk c-1
                        vector.wait_ge(d_sem, 16 * (2 * (c - 1) + m + 1))
                    vector.tensor_add(t_r[:, m, :], rP[m][:, 0, :],
                                      rP[m][:, 2, :]).then_inc(dve_sem, 1)

        @block.gpsimd
        def _(pool: bass.BassEngine):
            for c in range(NCH):
                for kt in range(KT):
                    if kt == 0 and c > 0:
                        # h18[m1][hi] WAR: PE read it for chunk c-1's L2
                        pool.wait_ge(mm_sem, mmT[("l2", c - 1, 1, MT - 1)])
                    pool.wait_ge(act_sem, actT[("r1", c, 1, kt)])
                    pool.tensor_copy(h18[:, 1, 0, kt, :], h1f[:, 1, kt, :]
                                     ).then_inc(pool_sem, 1)

    return nc


def get_nc():
    if "nc" not in _CACHE:
        _CACHE["nc"] = _build()
    return _CACHE["nc"]


def _split8(a, scale):
    """hi/lo fp8 split of scale*a."""
    s = a.astype(np.float32) * scale
    hi = s.astype(F8)
    lo = (s - hi.astype(np.float32)).astype(F8)
    return hi, lo


def _feat_major(a):
    # [K, F] -> [128, K//128, F]
    K_, F_ = a.shape
    return np.ascontiguousarray(
        a.reshape(K_ // 128, 128, F_).transpose(1, 0, 2))


def kernel(x, W1, b1, W2, b2, W3, b3):
    from concourse.bass_utils import run_bass_kernel_spmd

    nc = get_nc()
    x = np.asarray(x, dtype=np.float32)
    W1 = np.asarray(W1, dtype=np.float32)
    W2 = np.asarray(W2, dtype=np.float32)
    W3 = np.asarray(W3, dtype=np.float32)
    b1 = np.asarray(b1, dtype=np.float32)
    b2 = np.asarray(b2, dtype=np.float32)
    b3 = np.asarray(b3, dtype=np.float32)

    # x: [B, D] -> feature-major [128, KT, B], hi/lo split at 16x, then
    # chunk-contiguous [128, NCH, (2, KT, CH)]
    xT = np.ascontiguousarray(x.T)                    # [D, B]
    xhi, xlo = _split8(_feat_major(xT), SX)           # [128, KT, B] each
    xst = np.stack([xhi, xlo], axis=1)                # [128, 2, KT, B]
    xst = xst.reshape(128, 2, KT, NCH, CH)
    xd = np.ascontiguousarray(
        xst.transpose(0, 3, 1, 2, 4).reshape(128, NCH, 2 * KT * CH))

    def w_streams(Wm):
        # [D, H] -> [128, 2(lo,hi), KT, H] fp8 at 64x
        hi, lo = _split8(_feat_major(Wm), SW)
        return np.ascontiguousarray(np.stack([lo, hi], axis=1))

    def fm_small(v, scale=1.0):
        # [MPC, H] -> [128, MPC, H//128]
        return np.ascontiguousarray(
            (v * scale).reshape(MPC, H // 128, 128).transpose(2, 0, 1))

    in_maps = []
    for cidx in range(N_CORES):
        s = slice(MPC * cidx, MPC * (cidx + 1))
        im = {
            "xd": xd,
            "w3": fm_small(W3[s, :, 0]),
            "b1": fm_small(b1[s], SX),
            "b2": fm_small(b2[s]),
        }
        for m in range(MPC):
            im[f"w1_{m}"] = w_streams(W1[s][m])
            im[f"w2_{m}"] = w_streams(W2[s][m])
        in_maps.append(im)

    res = run_bass_kernel_spmd(nc, in_maps, list(range(N_CORES)))
    outs = []
    for r in res.results:
        # trd [128, NCH, MPC, CH] fp16 partial sums: finish the
        # 128-partition reduction on host
        t = np.asarray(r["trd"]).astype(np.float32).sum(axis=0)
        outs.append(t.transpose(1, 0, 2).reshape(MPC, B))  # [MPC, B]
    out = np.concatenate(outs, axis=0) + b3.reshape(E, 1)
    return out.reshape(E, B, 1).astype(np.float32)


# revision 3
# speedup vs baseline: 1.0200x; 1.0171x over previous
"""EnsembleFC (E=16 MLPs, 512->512->512->1, relu) on 8 TRN2 NeuronCores.

Expert parallel: each core owns E/8 = 2 members' weights and computes their
[B] output column; x replicated. Feature-major (transposed) activations:

    h1^T = relu(W1^T @ x^T + b1)      [H, B]
    h2^T = relu(W2^T @ h1^T + b2)     [H, B]
    out^T = W3^T @ h2^T               [1, B]   (b3 added on host)

L1/L2 run on the PE in fp8e4m3 DoubleRow perf mode (2 k-slots of 128 per
instruction at 0.5 cycles/moving-column -- 4x the f32r slot rate). Full
f32-level accuracy is kept with hi/lo splits: a = a_hi + a_lo (both fp8),
  a @ w = a_hi@w_hi + a_hi@w_lo + a_lo@w_hi   (+ a_lo@w_lo, dropped ~1e-4)
3 products at 4x rate = 1.33x over f32r (measured scaled err ~2e-3).
Scales keep fp8 in normal range: x,h1 carried at 16x, W1,W2 at 64x; the
act engine rescales while applying relu+bias. Per output tile the 6
DoubleRow matmuls are: hh(kt01), hh(kt23), then per-kt cross instructions
whose two slots pack (w_lo,a_hi)+(w_hi,a_lo).

x and W splits happen on host. h1's split is on-device, pipelined so the
PE never stalls (a PE gap costs ~3us of clock ramp in the p-state model):
  ACT:    h1f = relu(psA/64 + 16*b1) f32; h2 = relu(psB/1024 + b2) fp16
  DVE:    m0 hi=fp8(h1f), lo=fp8(h1f-hi); m1 lo; and the w3 reduction
          t_r[p,:] = sum_kt w3[p,kt]*h2[p,kt,:] as a mul/add tree in fp16
          (fp16 gets the DVE 2x/4x modes; scalar_tensor_tensor gets none)
  GPSIMD: m1 hi casts (all it supports: no PSUM access, tensor ops only)
L3: t_r partial sums are DMA'd straight to DRAM; the host finishes the
128-partition reduction (0.01% of the FLOPs). This keeps the whole L3
off the PE: a chunk is exactly 96 DoubleRow matmuls = 10.27us, and the
PE never waits on the reduction chain.

PSUM banks are mapped by output tile (bank=mt, psA for L1 / psB for L2),
so the only accumulation-bank WAR is against the other member's act
drain, half a chunk away. Per-chunk engine budgets: PE 10.27us, ACT 9.7,
DVE 9.6, GPSIMD 3.1.

Raw Bass (one wait per instruction), absolute semaphore tick tables.
"""
import numpy as np
import ml_dtypes

F8 = ml_dtypes.float8_e4m3

E, D, H, B = 16, 512, 512, 8192
N_CORES = 8
MPC = E // N_CORES          # members per core
KT = D // 128               # k-tiles per 512 contraction
MT = H // 128               # m-tiles per 512 output dim
CH = 512                    # batch columns per chunk (one psum bank)
NCH = B // CH               # chunks
XBUF = 4                    # x chunk buffering

SX = 16.0                   # x / h1 fp8 carry scale
SW = 64.0                   # W1 / W2 fp8 carry scale

_CACHE = {}

# L1 group order: m0's tiles early (its h1 chain gates L2-m0 at ~5.3us),
# m1 interleaved so the per-tile act->DVE chains keep up.
ORDER_L1 = [(0, 0), (0, 1), (0, 2), (1, 0), (0, 3), (1, 1), (1, 2), (1, 3)]
# L2 emission: all m0 then all m1 (m1's lo tiles land ~7.8us).
ORDER_L2 = [(0, 0), (0, 1), (0, 2), (0, 3), (1, 0), (1, 1), (1, 2), (1, 3)]


def _build():
    import concourse.bass as bass
    from concourse import mybir

    f32 = mybir.dt.float32
    f16 = mybir.dt.float16
    f8 = mybir.dt.float8e4
    DR = mybir.MatmulPerfMode.DoubleRow
    Relu = mybir.ActivationFunctionType.Relu

    nc = bass.Bass("TRN2", target_bir_lowering=False, debug=False,
                   num_devices=N_CORES)

    # dram (host pre-split/scaled; streams: x/h1 [hi,lo], w [lo,hi])
    xd = nc.dram_tensor("xd", [128, NCH, 2 * KT * CH], f8,
                        kind="ExternalInput").ap()
    w1 = [nc.dram_tensor(f"w1_{m}", [128, 2, KT, H], f8,
                         kind="ExternalInput").ap() for m in range(MPC)]
    w2 = [nc.dram_tensor(f"w2_{m}", [128, 2, KT, H], f8,
                         kind="ExternalInput").ap() for m in range(MPC)]
    w3 = nc.dram_tensor("w3", [128, MPC, KT], f32, kind="ExternalInput").ap()
    b1 = nc.dram_tensor("b1", [128, MPC, MT], f32, kind="ExternalInput").ap()
    b2 = nc.dram_tensor("b2", [128, MPC, MT], f32, kind="ExternalInput").ap()
    trd = nc.dram_tensor("trd", [128, NCH, MPC, CH], f16,
                         kind="ExternalOutput").ap()

    # sbuf
    w1s = [nc.alloc_sbuf_tensor(f"w1s{m}", [128, 2, KT, H], f8).ap()
           for m in range(MPC)]
    w2s = [nc.alloc_sbuf_tensor(f"w2s{m}", [128, 2, KT, H], f8).ap()
           for m in range(MPC)]
    w3s = nc.alloc_sbuf_tensor("w3s", [128, MPC, KT], f32).ap()
    b1s = nc.alloc_sbuf_tensor("b1s", [128, MPC, MT], f32).ap()
    b2s = nc.alloc_sbuf_tensor("b2s", [128, MPC, MT], f32).ap()
    xs = nc.alloc_sbuf_tensor("xs", [128, XBUF, 2, KT, CH], f8).ap()
    h1f = nc.alloc_sbuf_tensor("h1f", [128, MPC, MT, CH], f32).ap()
    h18 = nc.alloc_sbuf_tensor("h18", [128, MPC, 2, KT, CH], f8).ap()
    h2 = nc.alloc_sbuf_tensor("h2", [128, MPC, KT, CH], f16).ap()
    rP = [nc.alloc_sbuf_tensor(f"rP{m}", [128, KT, CH], f16).ap()
          for m in range(MPC)]
    rE = [nc.alloc_sbuf_tensor(f"rE{m}", [128, CH], f16).ap()
          for m in range(MPC)]
    rF = [nc.alloc_sbuf_tensor(f"rF{m}", [128, CH], f16).ap()
          for m in range(MPC)]
    t_r = nc.alloc_sbuf_tensor("t_r", [128, MPC, CH], f16).ap()

    psA = nc.alloc_psum_tensor("psA", [128, MT, CH], f32).ap()  # L1
    psB = nc.alloc_psum_tensor("psB", [128, MT, CH], f32).ap()  # L2

    # PE warmup scratch (uninitialized; the p-state model needs ~3us of
    # continuous execution to reach full clock)
    scr = nc.alloc_sbuf_tensor("scr", [128, 128 + CH],
                               mybir.dt.float32r).ap()
    N_WARM = _CACHE.get("n_warm_override", 8)
    N_WARM2 = _CACHE.get("n_warm2_override", 0)

    # --- tick tables (absolute counts, mirror emission order) ---
    mmT = {}
    _t = 0
    for c in range(NCH):
        for m, mt in ORDER_L1:
            _t += 1
            mmT[("l1", c, m, mt)] = _t
        for m, mt in ORDER_L2:
            _t += 1
            mmT[("l2", c, m, mt)] = _t

    actT = {}
    _a = 0
    for c in range(NCH):
        for m, mt in ORDER_L1:
            _a += 1
            actT[("r1", c, m, mt)] = _a
        for m, mt in ORDER_L2:
            _a += 1
            actT[("r2", c, m, mt)] = _a

    dveT = {}
    _d = 0
    for c in range(NCH):
        for kt in range(KT):
            _d += 1
            dveT[("hi", c, 0, kt)] = _d
            _d += 1
            dveT[("lo", c, 0, kt)] = _d
        for kt in range(KT):
            _d += 1
            dveT[("lo", c, 1, kt)] = _d
        for m in range(MPC):
            # mul0, mul1, addE, mul2, mul3, addF, t_r
            _d += 7
            dveT[("red", c, m)] = _d

    poolT = {}
    _p = 0
    for c in range(NCH):
        for kt in range(KT):
            _p += 1
            poolT[("hi", c, 1, kt)] = _p

    with (
        nc.Block() as block,
        nc.semaphore("mm_sem") as mm_sem,
        nc.semaphore("act_sem") as act_sem,
        nc.semaphore("b1_sem") as b1_sem,
        nc.semaphore("b2_sem") as b2_sem,
        nc.semaphore("w3_sem") as w3_sem,
        nc.semaphore("d_sem") as d_sem,
    ):
        x_sems = [nc.alloc_semaphore(f"x_sem{s}") for s in range(XBUF)]
        dve_sem = nc.alloc_semaphore("dve_sem")
        pool_sem = nc.alloc_semaphore("pool_sem")
        w1_sems = [nc.alloc_semaphore(f"w1_sem{m}") for m in range(MPC)]
        w2_sems = [nc.alloc_semaphore(f"w2_sem{m}") for m in range(MPC)]

        xs_flat = xs.rearrange("p s a b c -> p s (a b c)")

        def dma_x(sync, c):
            # whole chunk (both streams, all kt) in one contiguous DMA
            sync.dma_start(
                out=xs_flat[:, c % XBUF, :],
                in_=xd[:, c, :],
            ).then_inc(x_sems[c % XBUF], 16)

        @block.sync
        def _(sync: bass.BassEngine):
            def wflat(ap):
                return ap.rearrange("p s k h -> p (s k h)")

            # SP queue: member-0 W1 + x chunks; the rest loads in parallel
            # on the Activation queue. (One DMA each: the HWDGE descriptor
            # stage serializes at ~0.6us per DMA.)
            sync.dma_start(out=wflat(w1s[0]), in_=wflat(w1[0])
                           ).then_inc(w1_sems[0], 16)
            dma_x(sync, 0)
            dma_x(sync, 1)
            # x2/x3 after chunk 0 is underway: their transfers must not
            # queue ahead of w1s1/w2s on the serial DMA engine
            sync.wait_ge(mm_sem, 3)
            dma_x(sync, 2)
            dma_x(sync, 3)

            def dma_tr(cs, m):
                sync.wait_ge(dve_sem, dveT[("red", cs, m)])
                sync.dma_start(out=trd[:, cs, m, :], in_=t_r[:, m, :]
                               ).then_inc(d_sem, 16)

            for c in range(NCH):
                if c >= 1:
                    dma_tr(c - 1, 0)
                if c + XBUF < NCH:
                    lm, lmt = ORDER_L1[-1]
                    sync.wait_ge(mm_sem, mmT[("l1", c, lm, lmt)])
                    dma_x(sync, c + XBUF)
                if c >= 1:
                    dma_tr(c - 1, 1)
            for m in range(MPC):
                dma_tr(NCH - 1, m)
            sync.wait_ge(d_sem, 16 * MPC * NCH)

        @block.tensor
        def _(tensor: bass.BassEngine):
            for i in range(N_WARM):
                tensor.matmul(psA[:, 0, :], scr[:, :128], scr[:, 128:],
                              start=True, stop=True, skip_group_check=True)

            def dr_group(ps_bank, wsrc, asrc_hh, asrc_cr, waits=None):
                """6 DoubleRow matmuls accumulating one [128, CH] tile.

                waits: optional dict instr_idx -> (sem, tick) emitted
                before that instruction (0=hh01, 1=hh23, 2..5=cr kt)."""
                ins = None
                for idx in range(6):
                    if waits and idx in waits:
                        sem, tick = waits[idx]
                        tensor.wait_ge(sem, tick)
                    if idx < 2:
                        kt = 2 * idx
                        ins = tensor.matmul(
                            ps_bank, wsrc(1, slice(kt, kt + 2)),
                            asrc_hh(slice(kt, kt + 2)),
                            start=(idx == 0), stop=False, perf_mode=DR)
                    else:
                        kt = idx - 2
                        ins = tensor.matmul(
                            ps_bank, wsrc(slice(0, 2), kt), asrc_cr(kt),
                            start=False, stop=(idx == 5), perf_mode=DR)
                return ins

            for c in range(NCH):
                tensor.wait_ge(x_sems[c % XBUF], 16 * (c // XBUF + 1))
                for m, mt in ORDER_L1:
                    if c == 0:
                        if mt == 0:
                            tensor.wait_ge(w1_sems[m], 16)
                    else:
                        # psA bank=mt WAR vs the other member's r1 drain
                        om = 1 - m
                        cc = c - 1 if m == 0 else c
                        tensor.wait_ge(act_sem, actT[("r1", cc, om, mt)])
                    dr_group(
                        psA[:, mt, :],
                        lambda st, ktsl, m=m, cs=slice(mt * 128, (mt + 1) * 128):
                            w1s[m][:, st, ktsl, cs],
                        lambda ktsl, c=c: xs[:, c % XBUF, 0, ktsl, :],
                        lambda kt, c=c: xs[:, c % XBUF, :, kt, :],
                    ).then_inc(mm_sem, 1)

                if c == 0:
                    # keep the PE clock hot while the act/DVE pipeline
                    # fills for the first L2 phase
                    for i in range(N_WARM2):
                        tensor.matmul(psB[:, 0, :], scr[:, :128],
                                      scr[:, 128:], start=True, stop=True,
                                      skip_group_check=True)
                def l2_bank_wait(m, mt):
                    # psB bank=mt WAR vs the other member's r2 drain
                    if m == 0 and c >= 1:
                        tensor.wait_ge(act_sem, actT[("r2", c - 1, 1, mt)])
                    elif m == 1:
                        tensor.wait_ge(act_sem, actT[("r2", c, 0, mt)])

                def l2_w(m, mt, st, ktsl):
                    cs = slice(mt * 128, (mt + 1) * 128)
                    return w2s[m][:, st, ktsl, cs]

                for m in range(MPC):
                    if c == 0:
                        tensor.wait_ge(w2_sems[m], 16)
                    hisem, hiT = ((dve_sem, dveT) if m == 0 else
                                  (pool_sem, poolT))
                    # hh instructions of groups (m,0) and (m,1) first: they
                    # only need the hi stream, keeping the PE busy while
                    # the lo tiles land
                    l2_bank_wait(m, 0)
                    tensor.wait_ge(hisem, hiT[("hi", c, m, 1)])
                    tensor.matmul(psB[:, 0, :], l2_w(m, 0, 1, slice(0, 2)),
                                  h18[:, m, 0, 0:2, :],
                                  start=True, stop=False, perf_mode=DR)
                    tensor.wait_ge(hisem, hiT[("hi", c, m, 3)])
                    tensor.matmul(psB[:, 0, :], l2_w(m, 0, 1, slice(2, 4)),
                                  h18[:, m, 0, 2:4, :],
                                  start=False, stop=False, perf_mode=DR)
                    l2_bank_wait(m, 1)
                    for kt in (0, 2):
                        tensor.matmul(psB[:, 1, :], l2_w(m, 1, 1,
                                                         slice(kt, kt + 2)),
                                      h18[:, m, 0, kt:kt + 2, :],
                                      start=(kt == 0), stop=False,
                                      perf_mode=DR)
                    # cross instructions, group (m,0) then (m,1)
                    for mt in (0, 1):
                        ins = None
                        for kt in range(KT):
                            if mt == 0:
                                tensor.wait_ge(dve_sem,
                                               dveT[("lo", c, m, kt)])
                            ins = tensor.matmul(
                                psB[:, mt, :], l2_w(m, mt, slice(0, 2), kt),
                                h18[:, m, :, kt, :],
                                start=False, stop=(kt == KT - 1),
                                perf_mode=DR)
                        ins.then_inc(mm_sem, 1)
                    for mt in (2, 3):
                        l2_bank_wait(m, mt)
                        dr_group(
                            psB[:, mt, :],
                            lambda st, ktsl, m=m,
                                cs=slice(mt * 128, (mt + 1) * 128):
                                w2s[m][:, st, ktsl, cs],
                            lambda ktsl, m=m: h18[:, m, 0, ktsl, :],
                            lambda kt, m=m: h18[:, m, :, kt, :],
                        ).then_inc(mm_sem, 1)

        @block.scalar
        def _(scalar: bass.BassEngine):
            def wflat(ap):
                return ap.rearrange("p s k h -> p (s k h)")

            # prologue weight loads on the Activation HWDGE queue, in
            # parallel with SP's w1m0/x stream; tiny transfers first (the
            # DMA engine serializes transfers)
            scalar.dma_start(out=b1s, in_=b1).then_inc(b1_sem, 16)
            scalar.dma_start(out=b2s, in_=b2).then_inc(b2_sem, 16)
            scalar.dma_start(out=w3s, in_=w3).then_inc(w3_sem, 16)
            scalar.dma_start(out=wflat(w1s[1]), in_=wflat(w1[1])
                             ).then_inc(w1_sems[1], 16)
            scalar.dma_start(out=wflat(w2s[0]), in_=wflat(w2[0])
                             ).then_inc(w2_sems[0], 16)
            scalar.dma_start(out=wflat(w2s[1]), in_=wflat(w2[1])
                             ).then_inc(w2_sems[1], 16)
            scalar.wait_ge(b1_sem, 16)
            scalar.wait_ge(b2_sem, 16)
            for c in range(NCH):
                for m, mt in ORDER_L1:
                    if mt == 0 and c > 0:
                        # h1f[m] WAR: last lo pass of chunk c-1 done
                        scalar.wait_ge(dve_sem,
                                       dveT[("lo", c - 1, m, MT - 1)])
                    scalar.wait_ge(mm_sem, mmT[("l1", c, m, mt)])
                    # h1f = relu(psum/SW + SX*b1) = SX * h1_true
                    scalar.activation(
                        h1f[:, m, mt, :], psA[:, mt, :], Relu,
                        bias=b1s[:, m, mt:mt + 1], scale=1.0 / SW,
                    ).then_inc(act_sem, 1)
                for m, mt in ORDER_L2:
                    if mt == 0 and c > 0:
                        # h2[m] WAR: chunk c-1's reduction read it
                        scalar.wait_ge(dve_sem, dveT[("red", c - 1, m)])
                    scalar.wait_ge(mm_sem, mmT[("l2", c, m, mt)])
                    scalar.activation(
                        h2[:, m, mt, :], psB[:, mt, :], Relu,
                        bias=b2s[:, m, mt:mt + 1], scale=1.0 / (SX * SW),
                    ).then_inc(act_sem, 1)

        @block.vector
        def _(vector: bass.BassEngine):
            for c in range(NCH):
                # m0: hi + lo pairs per tile
                for kt in range(KT):
                    if kt == 0 and c > 0:
                        # h18[m0] WAR: PE read it for chunk c-1's L2
                        vector.wait_ge(mm_sem, mmT[("l2", c - 1, 0, MT - 1)])
                    vector.wait_ge(act_sem, actT[("r1", c, 0, kt)])
                    vector.tensor_copy(h18[:, 0, 0, kt, :], h1f[:, 0, kt, :]
                                       ).then_inc(dve_sem, 1)
                    vector.tensor_sub(h18[:, 0, 1, kt, :], h1f[:, 0, kt, :],
                                      h18[:, 0, 0, kt, :]).then_inc(dve_sem, 1)
                # m1: lo only (hi on gpsimd)
                for kt in range(KT):
                    vector.wait_ge(pool_sem, poolT[("hi", c, 1, kt)])
                    vector.tensor_sub(h18[:, 1, 1, kt, :], h1f[:, 1, kt, :],
                                      h18[:, 1, 0, kt, :]).then_inc(dve_sem, 1)
                # w3 reduction as fp16 muls (4x mode) + add tree (2x mode):
                # t_r[p,:] = sum_kt w3[p,kt] * h2[p,kt,:]
                if c == 0:
                    vector.wait_ge(w3_sem, 16)
                for m in range(MPC):
                    for kt in range(KT):
                        vector.wait_ge(act_sem, actT[("r2", c, m, kt)])
                        vector.tensor_scalar_mul(
                            rP[m][:, kt, :], h2[:, m, kt, :],
                            w3s[:, m, kt:kt + 1]).then_inc(dve_sem, 1)
                        if kt == 1:
                            vector.tensor_add(
                                rP[m][:, 0, :], rP[m][:, 0, :], rP[m][:, 1, :]
                            ).then_inc(dve_sem, 1)
                        if kt == 3:
                            vector.tensor_add(
                                rP[m][:, 2, :], rP[m][:, 2, :], rP[m][:, 3, :]
                            ).then_inc(dve_sem, 1)
                    if c >= 1:
                        # t_r[m] WAR vs its DMA of chunk c-1
                        vector.wait_ge(d_sem, 16 * (2 * (c - 1) + m + 1))
                    vector.tensor_add(t_r[:, m, :], rP[m][:, 0, :],
                                      rP[m][:, 2, :]).then_inc(dve_sem, 1)

        @block.gpsimd
        def _(pool: bass.BassEngine):
            for c in range(NCH):
                for kt in range(KT):
                    if kt == 0 and c > 0:
                        # h18[m1][hi] WAR: PE read it for chunk c-1's L2
                        pool.wait_ge(mm_sem, mmT[("l2", c - 1, 1, MT - 1)])
                    pool.wait_ge(act_sem, actT[("r1", c, 1, kt)])
                    pool.tensor_copy(h18[:, 1, 0, kt, :], h1f[:, 1, kt, :]
                                     ).then_inc(pool_sem, 1)

    return nc


def get_nc():
    if "nc" not in _CACHE:
        _CACHE["nc"] = _build()
    return _CACHE["nc"]


def _split8(a, scale):
    """hi/lo fp8 split of scale*a."""
    s = a.astype(np.float32) * scale
    hi = s.astype(F8)
    lo = (s - hi.astype(np.float32)).astype(F8)
    return hi, lo


def _feat_major(a):
    # [K, F] -> [128, K//128, F]
    K_, F_ = a.shape
    return np.ascontiguousarray(
        a.reshape(K_ // 128, 128, F_).transpose(1, 0, 2))


def kernel(x, W1, b1, W2, b2, W3, b3):
    from concourse.bass_utils import run_bass_kernel_spmd

    nc = get_nc()
    x = np.asarray(x, dtype=np.float32)
    W1 = np.asarray(W1, dtype=np.float32)
    W2 = np.asarray(W2, dtype=np.float32)
    W3 = np.asarray(W3, dtype=np.float32)
    b1 = np.asarray(b1, dtype=np.float32)
    b2 = np.asarray(b2, dtype=np.float32)
    b3 = np.asarray(b3, dtype=np.float32)

    # x: [B, D] -> feature-major [128, KT, B], hi/lo split at 16x, then
    # chunk-contiguous [128, NCH, (2, KT, CH)]
    xT = np.ascontiguousarray(x.T)                    # [D, B]
    xhi, xlo = _split8(_feat_major(xT), SX)           # [128, KT, B] each
    xst = np.stack([xhi, xlo], axis=1)                # [128, 2, KT, B]
    xst = xst.reshape(128, 2, KT, NCH, CH)
    xd = np.ascontiguousarray(
        xst.transpose(0, 3, 1, 2, 4).reshape(128, NCH, 2 * KT * CH))

    def w_streams(Wm):
        # [D, H] -> [128, 2(lo,hi), KT, H] fp8 at 64x
        hi, lo = _split8(_feat_major(Wm), SW)
        return np.ascontiguousarray(np.stack([lo, hi], axis=1))

    def fm_small(v, scale=1.0):
        # [MPC, H] -> [128, MPC, H//128]
        return np.ascontiguousarray(
            (v * scale).reshape(MPC, H // 128, 128).transpose(2, 0, 1))

    in_maps = []
    for cidx in range(N_CORES):
        s = slice(MPC * cidx, MPC * (cidx + 1))
        im = {
            "xd": xd,
            "w3": fm_small(W3[s, :, 0]),
            "b1": fm_small(b1[s], SX),
            "b2": fm_small(b2[s]),
        }
        for m in range(MPC):
            im[f"w1_{m}"] = w_streams(W1[s][m])
            im[f"w2_{m}"] = w_streams(W2[s][m])
        in_maps.append(im)

    res = run_bass_kernel_spmd(nc, in_maps, list(range(N_CORES)))
    outs = []
    for r in res.results:
        # trd [128, NCH, MPC, CH] fp16 partial sums: finish the
        # 128-partition reduction on host
        t = np.asarray(r["trd"]).astype(np.float32).sum(axis=0)
        outs.append(t.transpose(1, 0, 2).reshape(MPC, B))  # [MPC, B]
    out = np.concatenate(outs, axis=0) + b3.reshape(E, 1)
    return out.reshape(E, B, 1).astype(np.float32)


# revision 4
# speedup vs baseline: 1.0250x; 1.0049x over previous
"""EnsembleFC (E=16 MLPs, 512->512->512->1, relu) on 8 TRN2 NeuronCores.

Expert parallel: each core owns E/8 = 2 members' weights and computes their
[B] output column; x replicated. Feature-major (transposed) activations:

    h1^T = relu(W1^T @ x^T + b1)      [H, B]
    h2^T = relu(W2^T @ h1^T + b2)     [H, B]
    out^T = W3^T @ h2^T               [1, B]   (b3 added on host)

L1/L2 run on the PE in fp8e4m3 DoubleRow perf mode (2 k-slots of 128 per
instruction at 0.5 cycles/moving-column -- 4x the f32r slot rate). Full
f32-level accuracy is kept with hi/lo splits: a = a_hi + a_lo (both fp8),
  a @ w = a_hi@w_hi + a_hi@w_lo + a_lo@w_hi   (+ a_lo@w_lo, dropped ~1e-4)
3 products at 4x rate = 1.33x over f32r (measured scaled err ~2e-3).
Scales keep fp8 in normal range: x,h1 carried at 16x, W1,W2 at 64x; the
act engine rescales while applying relu+bias. Per output tile the 6
DoubleRow matmuls are: hh(kt01), hh(kt23), then per-kt cross instructions
whose two slots pack (w_lo,a_hi)+(w_hi,a_lo).

x and W splits happen on host. h1's split is on-device, pipelined so the
PE never stalls (a PE gap costs ~3us of clock ramp in the p-state model):
  ACT:    h1f = relu(psA/64 + 16*b1) f32; h2 = relu(psB/1024 + b2) fp16
  DVE:    m0 hi=fp8(h1f), lo=fp8(h1f-hi); m1 lo; and the w3 reduction
          t_r[p,:] = sum_kt w3[p,kt]*h2[p,kt,:] as a mul/add tree in fp16
          (fp16 gets the DVE 2x/4x modes; scalar_tensor_tensor gets none)
  GPSIMD: m1 hi casts (all it supports: no PSUM access, tensor ops only)
L3: t_r partial sums are DMA'd straight to DRAM; the host finishes the
128-partition reduction (0.01% of the FLOPs). This keeps the whole L3
off the PE: a chunk is exactly 96 DoubleRow matmuls = 10.27us, and the
PE never waits on the reduction chain.

PSUM banks are mapped by output tile (bank=mt, psA for L1 / psB for L2),
so the only accumulation-bank WAR is against the other member's act
drain, half a chunk away. Per-chunk engine budgets: PE 10.27us, ACT 9.7,
DVE 9.6, GPSIMD 3.1.

Raw Bass (one wait per instruction), absolute semaphore tick tables.
"""
import numpy as np
import ml_dtypes

F8 = ml_dtypes.float8_e4m3

E, D, H, B = 16, 512, 512, 8192
N_CORES = 8
MPC = E // N_CORES          # members per core
KT = D // 128               # k-tiles per 512 contraction
MT = H // 128               # m-tiles per 512 output dim
CH = 512                    # batch columns per chunk (one psum bank)
NCH = B // CH               # chunks
XBUF = 4                    # x chunk buffering

SX = 16.0                   # x / h1 fp8 carry scale
SW = 64.0                   # W1 / W2 fp8 carry scale

_CACHE = {}

# L1 group order: m0's tiles early (its h1 chain gates L2-m0 at ~5.3us),
# m1 interleaved so the per-tile act->DVE chains keep up.
ORDER_L1 = [(0, 0), (0, 1), (0, 2), (1, 0), (0, 3), (1, 1), (1, 2), (1, 3)]
# L2 emission: all m0 then all m1 (m1's lo tiles land ~7.8us).
ORDER_L2 = [(0, 0), (0, 1), (0, 2), (0, 3), (1, 0), (1, 1), (1, 2), (1, 3)]


def _build():
    import concourse.bass as bass
    from concourse import mybir

    f32 = mybir.dt.float32
    f16 = mybir.dt.float16
    f8 = mybir.dt.float8e4
    DR = mybir.MatmulPerfMode.DoubleRow
    Relu = mybir.ActivationFunctionType.Relu

    nc = bass.Bass("TRN2", target_bir_lowering=False, debug=False,
                   num_devices=N_CORES)

    # dram (host pre-split/scaled; streams: x/h1 [hi,lo], w [lo,hi])
    xd = nc.dram_tensor("xd", [128, NCH, 2 * KT * CH], f8,
                        kind="ExternalInput").ap()
    w1 = [nc.dram_tensor(f"w1_{m}", [128, 2, KT, H], f8,
                         kind="ExternalInput").ap() for m in range(MPC)]
    w2 = [nc.dram_tensor(f"w2_{m}", [128, 2, KT, H], f8,
                         kind="ExternalInput").ap() for m in range(MPC)]
    w3 = nc.dram_tensor("w3", [128, MPC, KT], f32, kind="ExternalInput").ap()
    b1 = nc.dram_tensor("b1", [128, MPC, MT], f32, kind="ExternalInput").ap()
    b2 = nc.dram_tensor("b2", [128, MPC, MT], f32, kind="ExternalInput").ap()
    trd = nc.dram_tensor("trd", [128, NCH, MPC, CH], f16,
                         kind="ExternalOutput").ap()
    # last chunk, member 1: the reduction tail would serialize behind the
    # final r2 drain; ship partials instead and let the host finish
    tp0 = nc.dram_tensor("tp0", [128, CH], f16, kind="ExternalOutput").ap()
    tp2 = nc.dram_tensor("tp2", [128, CH], f16, kind="ExternalOutput").ap()
    th3 = nc.dram_tensor("th3", [128, CH], f16, kind="ExternalOutput").ap()

    # sbuf
    w1s = [nc.alloc_sbuf_tensor(f"w1s{m}", [128, 2, KT, H], f8).ap()
           for m in range(MPC)]
    w2s = [nc.alloc_sbuf_tensor(f"w2s{m}", [128, 2, KT, H], f8).ap()
           for m in range(MPC)]
    w3s = nc.alloc_sbuf_tensor("w3s", [128, MPC, KT], f32).ap()
    b1s = nc.alloc_sbuf_tensor("b1s", [128, MPC, MT], f32).ap()
    b2s = nc.alloc_sbuf_tensor("b2s", [128, MPC, MT], f32).ap()
    xs = nc.alloc_sbuf_tensor("xs", [128, XBUF, 2, KT, CH], f8).ap()
    h1f = nc.alloc_sbuf_tensor("h1f", [128, MPC, MT, CH], f32).ap()
    h18 = nc.alloc_sbuf_tensor("h18", [128, MPC, 2, KT, CH], f8).ap()
    h2 = nc.alloc_sbuf_tensor("h2", [128, MPC, KT, CH], f16).ap()
    rP = [nc.alloc_sbuf_tensor(f"rP{m}", [128, KT, CH], f16).ap()
          for m in range(MPC)]
    rE = [nc.alloc_sbuf_tensor(f"rE{m}", [128, CH], f16).ap()
          for m in range(MPC)]
    rF = [nc.alloc_sbuf_tensor(f"rF{m}", [128, CH], f16).ap()
          for m in range(MPC)]
    t_r = nc.alloc_sbuf_tensor("t_r", [128, MPC, CH], f16).ap()

    psA = nc.alloc_psum_tensor("psA", [128, MT, CH], f32).ap()  # L1
    psB = nc.alloc_psum_tensor("psB", [128, MT, CH], f32).ap()  # L2

    # PE warmup scratch (uninitialized; the p-state model needs ~3us of
    # continuous execution to reach full clock)
    scr = nc.alloc_sbuf_tensor("scr", [128, 128 + CH],
                               mybir.dt.float32r).ap()
    N_WARM = _CACHE.get("n_warm_override", 8)
    N_WARM2 = _CACHE.get("n_warm2_override", 0)

    # --- tick tables (absolute counts, mirror emission order) ---
    mmT = {}
    _t = 0
    for c in range(NCH):
        for m, mt in ORDER_L1:
            _t += 1
            mmT[("l1", c, m, mt)] = _t
        for m, mt in ORDER_L2:
            _t += 1
            mmT[("l2", c, m, mt)] = _t

    actT = {}
    _a = 0
    for c in range(NCH):
        for m, mt in ORDER_L1:
            _a += 1
            actT[("r1", c, m, mt)] = _a
        for m, mt in ORDER_L2:
            _a += 1
            actT[("r2", c, m, mt)] = _a

    dveT = {}
    _d = 0
    for c in range(NCH):
        for kt in range(KT):
            _d += 1
            dveT[("hi", c, 0, kt)] = _d
            _d += 1
            dveT[("lo", c, 0, kt)] = _d
        for kt in range(KT):
            _d += 1
            dveT[("lo", c, 1, kt)] = _d
        for m in range(MPC):
            if c == NCH - 1 and m == 1:
                _d += 3
                dveT[("m1head",)] = _d
                _d += 1
                dveT[("m1mul2",)] = _d
            else:
                # mul0, mul1, addE, mul2, mul3, addF, t_r
                _d += 7
                dveT[("red", c, m)] = _d

    poolT = {}
    _p = 0
    for c in range(NCH):
        for kt in range(KT):
            _p += 1
            poolT[("hi", c, 1, kt)] = _p

    with (
        nc.Block() as block,
        nc.semaphore("mm_sem") as mm_sem,
        nc.semaphore("act_sem") as act_sem,
        nc.semaphore("b1_sem") as b1_sem,
        nc.semaphore("b2_sem") as b2_sem,
        nc.semaphore("w3_sem") as w3_sem,
        nc.semaphore("d_sem") as d_sem,
    ):
        x_sems = [nc.alloc_semaphore(f"x_sem{s}") for s in range(XBUF)]
        dve_sem = nc.alloc_semaphore("dve_sem")
        pool_sem = nc.alloc_semaphore("pool_sem")
        w1_sems = [nc.alloc_semaphore(f"w1_sem{m}") for m in range(MPC)]
        w2_sems = [nc.alloc_semaphore(f"w2_sem{m}") for m in range(MPC)]

        xs_flat = xs.rearrange("p s a b c -> p s (a b c)")

        def dma_x(sync, c):
            # whole chunk (both streams, all kt) in one contiguous DMA
            sync.dma_start(
                out=xs_flat[:, c % XBUF, :],
                in_=xd[:, c, :],
            ).then_inc(x_sems[c % XBUF], 16)

        @block.sync
        def _(sync: bass.BassEngine):
            def wflat(ap):
                return ap.rearrange("p s k h -> p (s k h)")

            # SP queue: member-0 W1 + x chunks; the rest loads in parallel
            # on the Activation queue. (One DMA each: the HWDGE descriptor
            # stage serializes at ~0.6us per DMA.)
            sync.dma_start(out=wflat(w1s[0]), in_=wflat(w1[0])
                           ).then_inc(w1_sems[0], 16)
            dma_x(sync, 0)
            # x1-x3 after chunk 0 is underway: their transfers must not
            # queue ahead of w1s1/w2s on the serial DMA engine
            sync.wait_ge(mm_sem, 1)
            dma_x(sync, 1)
            sync.wait_ge(mm_sem, 3)
            dma_x(sync, 2)
            dma_x(sync, 3)

            def dma_tr(cs, m):
                sync.wait_ge(dve_sem, dveT[("red", cs, m)])
                sync.dma_start(out=trd[:, cs, m, :], in_=t_r[:, m, :]
                               ).then_inc(d_sem, 16)

            for c in range(NCH):
                if c >= 1:
                    dma_tr(c - 1, 0)
                if c + XBUF < NCH:
                    lm, lmt = ORDER_L1[-1]
                    sync.wait_ge(mm_sem, mmT[("l1", c, lm, lmt)])
                    dma_x(sync, c + XBUF)
                if c >= 1:
                    dma_tr(c - 1, 1)
            dma_tr(NCH - 1, 0)
            sync.wait_ge(dve_sem, dveT[("m1head",)])
            sync.dma_start(out=tp0, in_=rP[1][:, 0, :]).then_inc(d_sem, 16)
            sync.wait_ge(dve_sem, dveT[("m1mul2",)])
            sync.dma_start(out=tp2, in_=rP[1][:, 2, :]).then_inc(d_sem, 16)
            sync.wait_ge(d_sem, 16 * (MPC * NCH + 2))

        @block.tensor
        def _(tensor: bass.BassEngine):
            for i in range(N_WARM):
                tensor.matmul(psA[:, 0, :], scr[:, :128], scr[:, 128:],
                              start=True, stop=True, skip_group_check=True)

            def dr_group(ps_bank, wsrc, asrc_hh, asrc_cr, waits=None):
                """6 DoubleRow matmuls accumulating one [128, CH] tile.

                waits: optional dict instr_idx -> (sem, tick) emitted
                before that instruction (0=hh01, 1=hh23, 2..5=cr kt)."""
                ins = None
                for idx in range(6):
                    if waits and idx in waits:
                        sem, tick = waits[idx]
                        tensor.wait_ge(sem, tick)
                    if idx < 2:
                        kt = 2 * idx
                        ins = tensor.matmul(
                            ps_bank, wsrc(1, slice(kt, kt + 2)),
                            asrc_hh(slice(kt, kt + 2)),
                            start=(idx == 0), stop=False, perf_mode=DR)
                    else:
                        kt = idx - 2
                        ins = tensor.matmul(
                            ps_bank, wsrc(slice(0, 2), kt), asrc_cr(kt),
                            start=False, stop=(idx == 5), perf_mode=DR)
                return ins

            for c in range(NCH):
                tensor.wait_ge(x_sems[c % XBUF], 16 * (c // XBUF + 1))
                for m, mt in ORDER_L1:
                    if c == 0:
                        if mt == 0:
                            tensor.wait_ge(w1_sems[m], 16)
                    else:
                        # psA bank=mt WAR vs the other member's r1 drain
                        om = 1 - m
                        cc = c - 1 if m == 0 else c
                        tensor.wait_ge(act_sem, actT[("r1", cc, om, mt)])
                    dr_group(
                        psA[:, mt, :],
                        lambda st, ktsl, m=m, cs=slice(mt * 128, (mt + 1) * 128):
                            w1s[m][:, st, ktsl, cs],
                        lambda ktsl, c=c: xs[:, c % XBUF, 0, ktsl, :],
                        lambda kt, c=c: xs[:, c % XBUF, :, kt, :],
                    ).then_inc(mm_sem, 1)

                if c == 0:
                    # keep the PE clock hot while the act/DVE pipeline
                    # fills for the first L2 phase
                    for i in range(N_WARM2):
                        tensor.matmul(psB[:, 0, :], scr[:, :128],
                                      scr[:, 128:], start=True, stop=True,
                                      skip_group_check=True)
                def l2_bank_wait(m, mt):
                    # psB bank=mt WAR vs the other member's r2 drain
                    if m == 0 and c >= 1:
                        tensor.wait_ge(act_sem, actT[("r2", c - 1, 1, mt)])
                    elif m == 1:
                        tensor.wait_ge(act_sem, actT[("r2", c, 0, mt)])

                def l2_w(m, mt, st, ktsl):
                    cs = slice(mt * 128, (mt + 1) * 128)
                    return w2s[m][:, st, ktsl, cs]

                for m in range(MPC):
                    if c == 0:
                        tensor.wait_ge(w2_sems[m], 16)
                    hisem, hiT = ((dve_sem, dveT) if m == 0 else
                                  (pool_sem, poolT))
                    # hh instructions of groups (m,0) and (m,1) first: they
                    # only need the hi stream, keeping the PE busy while
                    # the lo tiles land
                    l2_bank_wait(m, 0)
                    tensor.wait_ge(hisem, hiT[("hi", c, m, 1)])
                    tensor.matmul(psB[:, 0, :], l2_w(m, 0, 1, slice(0, 2)),
                                  h18[:, m, 0, 0:2, :],
                                  start=True, stop=False, perf_mode=DR)
                    tensor.wait_ge(hisem, hiT[("hi", c, m, 3)])
                    tensor.matmul(psB[:, 0, :], l2_w(m, 0, 1, slice(2, 4)),
                                  h18[:, m, 0, 2:4, :],
                                  start=False, stop=False, perf_mode=DR)
                    l2_bank_wait(m, 1)
                    for kt in (0, 2):
                        tensor.matmul(psB[:, 1, :], l2_w(m, 1, 1,
                                                         slice(kt, kt + 2)),
                                      h18[:, m, 0, kt:kt + 2, :],
                                      start=(kt == 0), stop=False,
                                      perf_mode=DR)
                    # cross instructions, group (m,0) then (m,1)
                    for mt in (0, 1):
                        ins = None
                        for kt in range(KT):
                            if mt == 0:
                                tensor.wait_ge(dve_sem,
                                               dveT[("lo", c, m, kt)])
                            ins = tensor.matmul(
                                psB[:, mt, :], l2_w(m, mt, slice(0, 2), kt),
                                h18[:, m, :, kt, :],
                                start=False, stop=(kt == KT - 1),
                                perf_mode=DR)
                        ins.then_inc(mm_sem, 1)
                    for mt in (2, 3):
                        l2_bank_wait(m, mt)
                        dr_group(
                            psB[:, mt, :],
                            lambda st, ktsl, m=m,
                                cs=slice(mt * 128, (mt + 1) * 128):
                                w2s[m][:, st, ktsl, cs],
                            lambda ktsl, m=m: h18[:, m, 0, ktsl, :],
                            lambda kt, m=m: h18[:, m, :, kt, :],
                        ).then_inc(mm_sem, 1)

        @block.scalar
        def _(scalar: bass.BassEngine):
            def wflat(ap):
                return ap.rearrange("p s k h -> p (s k h)")

            # prologue weight loads on the Activation HWDGE queue, in
            # parallel with SP's w1m0/x stream; tiny transfers first (the
            # DMA engine serializes transfers)
            scalar.dma_start(out=b1s, in_=b1).then_inc(b1_sem, 16)
            scalar.dma_start(out=b2s, in_=b2).then_inc(b2_sem, 16)
            scalar.dma_start(out=w3s, in_=w3).then_inc(w3_sem, 16)
            scalar.dma_start(out=wflat(w1s[1]), in_=wflat(w1[1])
                             ).then_inc(w1_sems[1], 16)
            scalar.dma_start(out=wflat(w2s[0]), in_=wflat(w2[0])
                             ).then_inc(w2_sems[0], 16)
            scalar.dma_start(out=wflat(w2s[1]), in_=wflat(w2[1])
                             ).then_inc(w2_sems[1], 16)
            scalar.wait_ge(b1_sem, 16)
            scalar.wait_ge(b2_sem, 16)
            for c in range(NCH):
                for m, mt in ORDER_L1:
                    if mt == 0 and c > 0:
                        # h1f[m] WAR: last lo pass of chunk c-1 done
                        scalar.wait_ge(dve_sem,
                                       dveT[("lo", c - 1, m, MT - 1)])
                    scalar.wait_ge(mm_sem, mmT[("l1", c, m, mt)])
                    # h1f = relu(psum/SW + SX*b1) = SX * h1_true
                    scalar.activation(
                        h1f[:, m, mt, :], psA[:, mt, :], Relu,
                        bias=b1s[:, m, mt:mt + 1], scale=1.0 / SW,
                    ).then_inc(act_sem, 1)
                for m, mt in ORDER_L2:
                    if mt == 0 and c > 0:
                        # h2[m] WAR: chunk c-1's reduction read it
                        scalar.wait_ge(dve_sem, dveT[("red", c - 1, m)])
                    scalar.wait_ge(mm_sem, mmT[("l2", c, m, mt)])
                    scalar.activation(
                        h2[:, m, mt, :], psB[:, mt, :], Relu,
                        bias=b2s[:, m, mt:mt + 1], scale=1.0 / (SX * SW),
                    ).then_inc(act_sem, 1)
                    if c == NCH - 1 and (m, mt) == (1, 3):
                        # ship the last h2 tile for the host-side w3 fold;
                        # act's DMA issue runs parallel to SP's queue
                        scalar.dma_start(out=th3, in_=h2[:, 1, 3, :]
                                         ).then_inc(d_sem, 16)

        @block.vector
        def _(vector: bass.BassEngine):
            for c in range(NCH):
                # m0: hi + lo pairs per tile
                for kt in range(KT):
                    if kt == 0 and c > 0:
                        # h18[m0] WAR: PE read it for chunk c-1's L2
                        vector.wait_ge(mm_sem, mmT[("l2", c - 1, 0, MT - 1)])
                    vector.wait_ge(act_sem, actT[("r1", c, 0, kt)])
                    vector.tensor_copy(h18[:, 0, 0, kt, :], h1f[:, 0, kt, :]
                                       ).then_inc(dve_sem, 1)
                    vector.tensor_sub(h18[:, 0, 1, kt, :], h1f[:, 0, kt, :],
                                      h18[:, 0, 0, kt, :]).then_inc(dve_sem, 1)
                # m1: lo only (hi on gpsimd)
                for kt in range(KT):
                    vector.wait_ge(pool_sem, poolT[("hi", c, 1, kt)])
                    vector.tensor_sub(h18[:, 1, 1, kt, :], h1f[:, 1, kt, :],
                                      h18[:, 1, 0, kt, :]).then_inc(dve_sem, 1)
                # w3 reduction as fp16 muls (4x mode) + add tree (2x mode):
                # t_r[p,:] = sum_kt w3[p,kt] * h2[p,kt,:]
                if c == 0:
                    vector.wait_ge(w3_sem, 16)
                for m in range(MPC):
                    last_m1 = (c == NCH - 1 and m == 1)
                    for kt in range(KT):
                        if last_m1 and kt == 3:
                            break   # host folds in w3*h2[kt3] from th3
                        vector.wait_ge(act_sem, actT[("r2", c, m, kt)])
                        vector.tensor_scalar_mul(
                            rP[m][:, kt, :], h2[:, m, kt, :],
                            w3s[:, m, kt:kt + 1]).then_inc(dve_sem, 1)
                        if kt == 1:
                            vector.tensor_add(
                                rP[m][:, 0, :], rP[m][:, 0, :], rP[m][:, 1, :]
                            ).then_inc(dve_sem, 1)
                        if kt == 3:
                            vector.tensor_add(
                                rP[m][:, 2, :], rP[m][:, 2, :], rP[m][:, 3, :]
                            ).then_inc(dve_sem, 1)
                    if last_m1:
                        continue
                    if c >= 1:
                        # t_r[m] WAR vs its DMA of chunk c-1
                        vector.wait_ge(d_sem, 16 * (2 * (c - 1) + m + 1))
                    vector.tensor_add(t_r[:, m, :], rP[m][:, 0, :],
                                      rP[m][:, 2, :]).then_inc(dve_sem, 1)

        @block.gpsimd
        def _(pool: bass.BassEngine):
            for c in range(NCH):
                for kt in range(KT):
                    if kt == 0 and c > 0:
                        # h18[m1][hi] WAR: PE read it for chunk c-1's L2
                        pool.wait_ge(mm_sem, mmT[("l2", c - 1, 1, MT - 1)])
                    pool.wait_ge(act_sem, actT[("r1", c, 1, kt)])
                    pool.tensor_copy(h18[:, 1, 0, kt, :], h1f[:, 1, kt, :]
                                     ).then_inc(pool_sem, 1)

    return nc


def get_nc():
    if "nc" not in _CACHE:
        _CACHE["nc"] = _build()
    return _CACHE["nc"]


def _split8(a, scale):
    """hi/lo fp8 split of scale*a."""
    s = a.astype(np.float32) * scale
    hi = s.astype(F8)
    lo = (s - hi.astype(np.float32)).astype(F8)
    return hi, lo


def _feat_major(a):
    # [K, F] -> [128, K//128, F]
    K_, F_ = a.shape
    return np.ascontiguousarray(
        a.reshape(K_ // 128, 128, F_).transpose(1, 0, 2))


def kernel(x, W1, b1, W2, b2, W3, b3):
    from concourse.bass_utils import run_bass_kernel_spmd

    nc = get_nc()
    x = np.asarray(x, dtype=np.float32)
    W1 = np.asarray(W1, dtype=np.float32)
    W2 = np.asarray(W2, dtype=np.float32)
    W3 = np.asarray(W3, dtype=np.float32)
    b1 = np.asarray(b1, dtype=np.float32)
    b2 = np.asarray(b2, dtype=np.float32)
    b3 = np.asarray(b3, dtype=np.float32)

    # x: [B, D] -> feature-major [128, KT, B], hi/lo split at 16x, then
    # chunk-contiguous [128, NCH, (2, KT, CH)]
    xT = np.ascontiguousarray(x.T)                    # [D, B]
    xhi, xlo = _split8(_feat_major(xT), SX)           # [128, KT, B] each
    xst = np.stack([xhi, xlo], axis=1)                # [128, 2, KT, B]
    xst = xst.reshape(128, 2, KT, NCH, CH)
    xd = np.ascontiguousarray(
        xst.transpose(0, 3, 1, 2, 4).reshape(128, NCH, 2 * KT * CH))

    def w_streams(Wm):
        # [D, H] -> [128, 2(lo,hi), KT, H] fp8 at 64x
        hi, lo = _split8(_feat_major(Wm), SW)
        return np.ascontiguousarray(np.stack([lo, hi], axis=1))

    def fm_small(v, scale=1.0):
        # [MPC, H] -> [128, MPC, H//128]
        return np.ascontiguousarray(
            (v * scale).reshape(MPC, H // 128, 128).transpose(2, 0, 1))

    in_maps = []
    for cidx in range(N_CORES):
        s = slice(MPC * cidx, MPC * (cidx + 1))
        im = {
            "xd": xd,
            "w3": fm_small(W3[s, :, 0]),
            "b1": fm_small(b1[s], SX),
            "b2": fm_small(b2[s]),
        }
        for m in range(MPC):
            im[f"w1_{m}"] = w_streams(W1[s][m])
            im[f"w2_{m}"] = w_streams(W2[s][m])
        in_maps.append(im)

    res = run_bass_kernel_spmd(nc, in_maps, list(range(N_CORES)))
    outs = []
    for cidx, r in enumerate(res.results):
        # trd [128, NCH, MPC, CH] fp16 partial sums: finish the
        # 128-partition reduction on host
        t = np.asarray(r["trd"]).astype(np.float32).sum(axis=0)
        t = t.transpose(1, 0, 2).reshape(MPC, B)           # [MPC, B]
        # last chunk, member 1 arrived as pieces
        w3c = W3[MPC * cidx + 1, 3 * 128:4 * 128, 0]       # [128]
        piece = (np.asarray(r["tp0"]).astype(np.float32)
                 + np.asarray(r["tp2"]).astype(np.float32)
                 + w3c[:, None] * np.asarray(r["th3"]).astype(np.float32))
        t[1, (NCH - 1) * CH:] = piece.sum(axis=0)
        outs.append(t)
    out = np.concatenate(outs, axis=0) + b3.reshape(E, 1)
    return out.reshape(E, B, 1).astype(np.float32)
